# revision 4
# baseline (speedup 1.0000x reference)
"""GAT layer (nn_GATLayer_44220983279640) — Trainium2 Bass/Tile kernel.

Reference math per graph (B=16, D=512, FIN=FOUT=128, H=8):
    h  = x @ W                                         [D, F]
    s1[hd,i] = h[i] . a1[hd]   s2[hd,j] = h[j] . a2[hd]
    e  = leaky_relu(s1[:,None] + s2[None,:] + ab)      [H, D, D]
    att = softmax_j(where(adj > 0, e, -9e15))
    out = mean_hd(att @ h)                             [D, F]

Sharding: data-parallel over batch, 2 graphs per core on 8 cores.

Device strategy v2 (all-fp16 elementwise path):
  * E^T[j, i] layout; additive fp16 mask (adj>0 ? 0 : -6e4) prepared host-side.
  * v = (maskT + s2[j]) + s1b on DVE as 4 chunk STTs, all operands fp16 so
    the DVE runs 2x_1P mode (halves the f32 baseline cost).
  * leaky_relu on DVE as ONE wide STT: u = max(v*0.01, v) — frees an entire
    ACT pass per head-graph vs Prelu on ACT.
  * exp on ACT, one wide [128, 2048] fp16 pass, shifted per head so outputs
    are in (0, e^8] (fp16-normal); shift errors cancel in softmax exactly.
  * agg matmuls: psum[i-tile] += E^T-slice^T @ [h/8 | ones] (fp16 weights).
  * normalize-evict on ACT: Copy(psum * rcol) with per-partition scale AP;
    head-accumulate as ONE wide fp16 tensor_tensor add on DVE.
  * fp16 setup matmuls (1 cyc/row vs 4 for f32), one batched mask DMA per
    graph, fp16 output (host upcasts to f32).
"""

from contextlib import ExitStack

import numpy as np

import concourse.bass as bass
import concourse.bacc as bacc
import concourse.tile as tile
from concourse import mybir
from concourse.bass_utils import run_bass_kernel_spmd

B, D, FIN, FOUT, H = 16, 512, 128, 128, 8
NCORES = 8
NB = B // NCORES          # graphs per core
P = 128                   # partitions
NCH = D // P              # 4 j-chunks / i-tiles
NEGM = -60000.0           # fp16-safe "masked" logit

F32 = mybir.dt.float32
F16 = mybir.dt.float16

# packed fp16 consts layout (columns): W | W^T | aT | ab | id8 | ones | zero
C_W = 0
C_WT = FOUT
C_AT = 2 * FOUT
C_AB = 2 * FOUT + 2 * H
C_ID8 = C_AB + 1
C_ONES = C_ID8 + H
C_ZERO = C_ONES + P
CONST_COLS = C_ZERO + 1

_NC_CACHE = {}


def _build_bass():
    nc = bacc.Bacc("TRN2", debug=False, num_devices=NCORES)

    xT = nc.dram_tensor("xT", [NB, FIN, D], F16, kind="ExternalInput").ap()
    maskT = nc.dram_tensor("maskT", [NB, P, NCH * D], F16, kind="ExternalInput").ap()
    consts = nc.dram_tensor("consts", [P, CONST_COLS], F16, kind="ExternalInput").ap()
    s1d = nc.dram_tensor("s1d", [NB, H, D], F16).ap()
    out = nc.dram_tensor("out", [NB, D, FOUT], F16, kind="ExternalOutput").ap()

    with tile.TileContext(nc) as tc, ExitStack() as ctx:
        _kernel_body(ctx, tc, out, xT, maskT, consts, s1d)
    nc.compile()
    return nc


def _kernel_body(ctx, tc, out, xT, maskT, consts, s1d):
    nc = tc.nc
    add, mult, amax = mybir.AluOpType.add, mybir.AluOpType.mult, mybir.AluOpType.max

    const = ctx.enter_context(tc.tile_pool(name="const", bufs=1))
    xpool = ctx.enter_context(tc.tile_pool(name="xpool", bufs=NB))
    mpool = ctx.enter_context(tc.tile_pool(name="mpool", bufs=NB))
    spool = ctx.enter_context(tc.tile_pool(name="spool", bufs=NB))
    s2tpool = ctx.enter_context(tc.tile_pool(name="s2tpool", bufs=2 * NCH))
    vpool = ctx.enter_context(tc.tile_pool(name="vpool", bufs=4))
    upool = ctx.enter_context(tc.tile_pool(name="upool", bufs=4))
    epool = ctx.enter_context(tc.tile_pool(name="epool", bufs=4))
    s1bpool = ctx.enter_context(tc.tile_pool(name="s1bpool", bufs=6))
    hpool = ctx.enter_context(tc.tile_pool(name="hpool", bufs=2 * NCH))
    apool = ctx.enter_context(tc.tile_pool(name="apool", bufs=2))
    evpool = ctx.enter_context(tc.tile_pool(name="evpool", bufs=4))
    rpool = ctx.enter_context(tc.tile_pool(name="rpool", bufs=12))
    # PSUM: 2 (setup scratch) + 6 (agg out) = 8 banks
    pset = ctx.enter_context(tc.tile_pool(name="pset", bufs=2, space="PSUM"))
    pout = ctx.enter_context(tc.tile_pool(name="pout", bufs=6, space="PSUM"))

    # --- constants (one packed DMA) ----------------------------------------
    cst = const.tile([P, CONST_COLS], F16)
    nc.sync.dma_start(out=cst, in_=consts)
    W_sb = cst[:, C_W : C_W + FOUT]
    WT_sb = cst[:, C_WT : C_WT + FOUT]
    aT_sb = cst[:, C_AT : C_AT + 2 * H]
    ab_sb = cst[0:H, C_AB : C_AB + 1]
    ident8 = cst[0:H, C_ID8 : C_ID8 + H]
    onesrow = cst[0:1, C_ONES : C_ONES + P]

    # Wa[fin, 0:8]=W@a1^T, [fin, 8:16]=W@a2^T  (shared across graphs)
    p_wa = pset.tile([P, D], F32, tag="setup")
    nc.tensor.matmul(p_wa[:, 0 : 2 * H], WT_sb, aT_sb, start=True, stop=True)
    Wa_sb = const.tile([FIN, 2 * H], F16)
    nc.scalar.activation(Wa_sb[:], p_wa[:, 0 : 2 * H], mybir.ActivationFunctionType.Copy)

    G = []  # per-graph setup state
    for b in range(NB):
        # --- per-graph setup ----------------------------------------------
        x_sb = xpool.tile([FIN, D], F16, tag="x")
        nc.sync.dma_start(out=x_sb, in_=xT[b])
        m_sb = mpool.tile([P, NCH * D], F16, tag="mask")
        nc.gpsimd.dma_start(out=m_sb, in_=maskT[b])

        # s1/s2 for all heads: [8, D] each
        p_s1 = pset.tile([P, D], F32, tag="setup")
        nc.tensor.matmul(p_s1[0:H, :], Wa_sb[:, 0:H], x_sb[:], start=True, stop=True)
        s1_sb = spool.tile([H, D], F16, tag="s1")
        nc.scalar.activation(s1_sb[:], p_s1[0:H, :], mybir.ActivationFunctionType.Copy)
        # stage s1 rows in DRAM; the head loop row-broadcasts them back via DMA
        nc.scalar.dma_start(out=s1d[b], in_=s1_sb[:])
        mx1 = spool.tile([H, 1], F32, tag="mx1")
        nc.vector.reduce_max(
            out=mx1[:], in_=p_s1[0:H, :], axis=mybir.AxisListType.X, negate=True
        )

        p_s2 = pset.tile([P, D], F32, tag="setup")
        nc.tensor.matmul(
            p_s2[0:H, :], Wa_sb[:, H : 2 * H], x_sb[:], start=True, stop=True
        )
        s2b_sb = spool.tile([H, D], F16, tag="s2")
        nc.scalar.activation(
            s2b_sb[:], p_s2[0:H, :], mybir.ActivationFunctionType.Identity,
            bias=ab_sb,
        )
        mx2 = spool.tile([H, 1], F32, tag="mx2")
        nc.vector.reduce_max(
            out=mx2[:], in_=s2b_sb[:], axis=mybir.AxisListType.X, negate=True
        )

        # Per-head negated logit upper bound + 8: exp bias (softmax-shift
        # errors cancel per head, so fp16 is fine here).
        nbound = spool.tile([H, 1], F32, tag="nbound")
        nc.vector.tensor_add(nbound[:], mx1[:], mx2[:])
        nc.vector.tensor_scalar_add(nbound[:], nbound[:], 8.0)
        nb16 = spool.tile([H, 1], F16, tag="nb16")
        nc.vector.tensor_copy(nb16[:], nbound[:])
        # broadcast -bound to [P, H] columns: transpose to a row, then
        # ones-row outer-product
        p_nt = pset.tile([P, D], F32, tag="setup")
        nc.tensor.matmul(p_nt[0:1, 0:H], nb16[:], ident8, start=True, stop=True)
        nbT = spool.tile([1, H], F16, tag="nbT")
        nc.vector.tensor_copy(nbT[:], p_nt[0:1, 0:H])
        p_nb = pset.tile([P, D], F32, tag="setup")
        nc.tensor.matmul(p_nb[:, 0:H], onesrow, nbT[:], start=True, stop=True)
        nbcols = spool.tile([P, H], F16, tag="nbcols")
        nc.scalar.activation(
            nbcols[:], p_nb[:, 0:H], mybir.ActivationFunctionType.Copy
        )

        # s2b columns: [P, H] per j-chunk (PE transpose of [8, 128] slices)
        s2bT = []
        for c in range(NCH):
            p_t = pset.tile([P, 2 * D], F16, tag="setup")
            nc.tensor.transpose(p_t[:, 0:H], s2b_sb[:, bass.ts(c, P)], ident8)
            st = s2tpool.tile([P, H], F16, tag="s2T")
            nc.scalar.activation(st[:], p_t[:, 0:H], mybir.ActivationFunctionType.Copy)
            s2bT.append(st)

        # h tiles + ones column, fp16, h pre-scaled by 1/H
        haug = []
        for c in range(NCH):
            p_h = pset.tile([P, D], F32, tag="setup")
            nc.tensor.matmul(
                p_h[:, 0:FOUT], x_sb[:, bass.ts(c, P)], W_sb, start=True, stop=True
            )
            ha = hpool.tile([P, FOUT + 1], F16, tag="haug")
            nc.scalar.activation(
                ha[:, 0:FOUT], p_h[:, 0:FOUT],
                mybir.ActivationFunctionType.Copy, scale=1.0 / H,
            )
            nc.vector.memset(ha[:, FOUT : FOUT + 1], 1.0)
            haug.append(ha)

        acc = apool.tile([P, NCH * FOUT], F16, tag="acc")
        G.append(dict(m_sb=m_sb, s2bT=s2bT, haug=haug, acc=acc, nbcols=nbcols))

    # --- main per-head loop, graphs interleaved for deeper ILP ------------
    for hd in range(H):
        for b in range(NB):
            m_sb, s2bT = G[b]["m_sb"], G[b]["s2bT"]
            haug, acc, nbcols = G[b]["haug"], G[b]["acc"], G[b]["nbcols"]
            # S1B = s1 row hd broadcast across partitions (DMA row-bcast)
            s1b = s1bpool.tile([P, D], F16, tag="s1b")
            s1row = s1d[b, hd]
            nc.gpsimd.dma_start(
                out=s1b[:],
                in_=bass.AP(
                    tensor=s1d.tensor, offset=s1row.offset,
                    ap=[[0, P], s1row.ap[-1]],
                ),
            )

            # v = (maskT + s2b[j]) + S1B   (fp16 STT -> DVE 2x mode)
            v = vpool.tile([P, NCH * D], F16, tag="v")
            for c in range(NCH):
                nc.vector.scalar_tensor_tensor(
                    out=v[:, bass.ts(c, D)],
                    in0=m_sb[:, bass.ts(c, D)],
                    scalar=s2bT[c][:, hd : hd + 1],
                    in1=s1b[:],
                    op0=add,
                    op1=add,
                )

            # u = leaky_relu(v) = max(0.01*v, v), one wide fp16 STT on DVE
            u = upool.tile([P, NCH * D], F16, tag="u")
            nc.vector.scalar_tensor_tensor(
                out=u[:], in0=v[:], scalar=0.01, in1=v[:], op0=mult, op1=amax,
            )
            # E = exp(u - bound + 8), one wide fp16 ACT pass
            E = epool.tile([P, NCH * D], F16, tag="E")
            nc.scalar.activation(
                E[:], u[:], mybir.ActivationFunctionType.Exp,
                bias=nbcols[:, hd : hd + 1],
            )

            # agg: psum[i-tile t] += E^T[:, t]^T @ [h/8 | 1]
            p_os, rcols = [], []
            for t in range(NCH):
                p_o = pout.tile([P, FOUT + 1], F32, tag="po")
                for c in range(NCH):
                    nc.tensor.matmul(
                        p_o[:],
                        E[:, c * D + t * P : c * D + (t + 1) * P],
                        haug[c][:],
                        start=(c == 0),
                        stop=(c == NCH - 1),
                    )
                p_os.append(p_o)
            for t in range(NCH):
                rcol = rpool.tile([P, 1], F32, tag="rcol")
                nc.vector.reciprocal(rcol[:], p_os[t][:, FOUT : FOUT + 1])
                rcols.append(rcol)
            # normalize-evict on ACT (Copy with per-partition scale AP);
            # first head writes acc directly, later heads go via ev + one
            # wide fp16 DVE add.
            if hd == 0:
                for t in range(NCH):
                    nc.scalar.activation(
                        acc[:, bass.ts(t, FOUT)], p_os[t][:, 0:FOUT],
                        mybir.ActivationFunctionType.Copy, scale=rcols[t][:],
                    )
            else:
                ev = evpool.tile([P, NCH * FOUT], F16, tag="ev")
                for t in range(NCH):
                    nc.scalar.activation(
                        ev[:, bass.ts(t, FOUT)], p_os[t][:, 0:FOUT],
                        mybir.ActivationFunctionType.Copy, scale=rcols[t][:],
                    )
                nc.vector.tensor_add(acc[:], acc[:], ev[:])

    for b in range(NB):
        # out[b, t*128+p, f] = acc[p, t*FOUT+f]
        nc.sync.dma_start(
            out=bass.AP(
                tensor=out.tensor, offset=out[b].offset,
                ap=[[FOUT, P], [P * FOUT, NCH], [1, FOUT]],
            ),
            in_=G[b]["acc"][:],
        )


def _prep_core_inputs(input, adj, W, a_w, a_b, core):
    gs = slice(core * NB, (core + 1) * NB)
    x_c = input[gs]                                   # [NB, D, FIN]
    adj_c = adj[gs]                                   # [NB, D, D] int32
    xT = np.ascontiguousarray(x_c.transpose(0, 2, 1)).astype(np.float16)
    adjT = adj_c.transpose(0, 2, 1)                   # [NB, j, i]

    maskT = np.where(adjT > 0, np.float16(0.0), np.float16(NEGM))
    # [NB, j, i] -> [NB, NCH, P, i] -> [NB, P, NCH, i] -> [NB, P, NCH*D]
    maskT = np.ascontiguousarray(
        maskT.reshape(NB, NCH, P, D).transpose(0, 2, 1, 3).reshape(NB, P, NCH * D)
    )
    return {
        "xT": xT,
        "maskT": maskT,
        "consts": _pack_consts(W, a_w, a_b),
    }


def _pack_consts(W, a_w, a_b):
    c = np.zeros((P, CONST_COLS), dtype=np.float16)
    c[:, C_W : C_W + FOUT] = W
    c[:, C_WT : C_WT + FOUT] = W.T
    c[:, C_AT : C_AT + H] = a_w[:, :FOUT].T
    c[:, C_AT + H : C_AT + 2 * H] = a_w[:, FOUT:].T
    c[0:H, C_AB] = a_b
    c[0:H, C_ID8 : C_ID8 + H] = np.eye(H)
    c[0:1, C_ONES : C_ONES + P] = 1.0
    return c


def get_nc():
    if "nc" not in _NC_CACHE:
        _NC_CACHE["nc"] = _build_bass()
    return _NC_CACHE["nc"]


def run_on_device(in_maps, **kwargs):
    return run_bass_kernel_spmd(get_nc(), in_maps, list(range(NCORES)), **kwargs)


def kernel(input, adj, W, a_w, a_b):
    input = np.asarray(input, dtype=np.float32)
    adj = np.asarray(adj)
    W = np.asarray(W, dtype=np.float32)
    a_w = np.asarray(a_w, dtype=np.float32)
    a_b = np.asarray(a_b, dtype=np.float32)

    in_maps = [
        _prep_core_inputs(input, adj, W, a_w, a_b, c) for c in range(NCORES)
    ]
    res = run_on_device(in_maps)
    outs = [res.results[c]["out"] for c in range(NCORES)]
    return np.concatenate(outs, axis=0).astype(np.float32)


if __name__ == "__main__":
    nc = get_nc()
    print("built ok")


# revision 8
# speedup vs baseline: 1.2574x; 1.2574x over previous
"""GAT layer (nn_GATLayer_44220983279640) — Trainium2 Bass/Tile kernel.

Reference math per graph (B=16, D=512, FIN=FOUT=128, H=8):
    h  = x @ W                                         [D, F]
    s1[hd,i] = h[i] . a1[hd]   s2[hd,j] = h[j] . a2[hd]
    e  = leaky_relu(s1[:,None] + s2[None,:] + ab)      [H, D, D]
    att = softmax_j(where(adj > 0, e, -9e15))
    out = mean_hd(att @ h)                             [D, F]

Sharding: data-parallel over batch, 2 graphs per core on 8 cores.

Device strategy v2 (all-fp16 elementwise path):
  * E^T[j, i] layout; additive fp16 mask (adj>0 ? 0 : -6e4) prepared host-side.
  * v = (maskT + s2[j]) + s1b on DVE as 4 chunk STTs, all operands fp16 so
    the DVE runs 2x_1P mode (halves the f32 baseline cost).
  * leaky_relu on DVE as ONE wide STT: u = max(v*0.01, v) — frees an entire
    ACT pass per head-graph vs Prelu on ACT.
  * exp on ACT, one wide [128, 2048] fp16 pass, shifted per head so outputs
    are in (0, e^8] (fp16-normal); shift errors cancel in softmax exactly.
  * agg matmuls: psum[i-tile] += E^T-slice^T @ [h/8 | ones] (fp16 weights).
  * normalize-evict on ACT: Copy(psum * rcol) with per-partition scale AP;
    head-accumulate as ONE wide fp16 tensor_tensor add on DVE.
  * fp16 setup matmuls (1 cyc/row vs 4 for f32), one batched mask DMA per
    graph, fp16 output (host upcasts to f32).
"""

from contextlib import ExitStack

import numpy as np

import concourse.bass as bass
import concourse.bacc as bacc
import concourse.tile as tile
from concourse import mybir
from concourse import dve_ops as _dvo
from concourse.bass_utils import run_bass_kernel_spmd
from concourse.dve_spec import C0, C2, Spec, Src0, Src1, lower, maxx
from concourse.dve_uop import DveOpSpec


def _register_vlrelu():
    """Custom fused DVE op: out = leaky_relu(in0 + s0 + in1).

    One 1x DVE pass replaces the STT (mask + s2 + s1b) AND the leaky_relu
    pass (scalar_tensor_tensor has no 2x uop, so two stock passes would cost
    ~2x this single fused op)."""
    name = "GAT_VLRELU_ANT"
    for op in _dvo.OPS:
        if op.name == name:
            return op
    x = (Src0 + C0) + Src1

    def _ref(in0, in1, c0, c1, c2):
        y = in0.astype(np.float32) + c0 + in1.astype(np.float32)
        return np.maximum(y, y * c2)

    spec = Spec(body=maxx(x, x * C2), reference=_ref)
    row = _dvo._CUSTOM_DVE_ROW_BASE + len(_dvo.OPS)
    shas = {}
    for ver in ("v3", "v4"):
        try:
            uops = lower(spec, ver=ver)
            shas[ver] = DveOpSpec(
                name=name, opcode=row, uops=uops, rd1_en=True
            ).sha(ver)
        except Exception:
            pass
    op = _dvo.DveOp(name, spec, subdim=False, uops_sha=shas)
    _dvo.OPS.append(op)
    _dvo._SUB_OPCODE_FOR_NAME[name] = row
    return op


VLRELU = _register_vlrelu()

B, D, FIN, FOUT, H = 16, 512, 128, 128, 8
NCORES = 8
NB = B // NCORES          # graphs per core
P = 128                   # partitions
NCH = D // P              # 4 j-chunks / i-tiles
NEGM = -60000.0           # fp16-safe "masked" logit

F32 = mybir.dt.float32
F16 = mybir.dt.float16

# packed fp16 consts layout (columns): W | W^T | aT | ab | id8 | ones | zero
C_W = 0
C_WT = FOUT
C_AT = 2 * FOUT
C_AB = 2 * FOUT + 2 * H
C_ID8 = C_AB + 1
C_ONES = C_ID8 + H
C_ZERO = C_ONES + P
CONST_COLS = C_ZERO + 1

_NC_CACHE = {}


def _build_bass():
    nc = bacc.Bacc("TRN2", debug=False, num_devices=NCORES)

    xT = nc.dram_tensor("xT", [NB, FIN, D], F16, kind="ExternalInput").ap()
    maskT = nc.dram_tensor("maskT", [NB, P, NCH * D], F16, kind="ExternalInput").ap()
    consts = nc.dram_tensor("consts", [P, CONST_COLS], F16, kind="ExternalInput").ap()
    s1d = nc.dram_tensor("s1d", [NB, H, D], F16).ap()
    out = nc.dram_tensor("out", [NB, D, FOUT], F16, kind="ExternalOutput").ap()

    with tile.TileContext(nc) as tc, ExitStack() as ctx:
        _kernel_body(ctx, tc, out, xT, maskT, consts, s1d)
    nc.compile()
    return nc


def _kernel_body(ctx, tc, out, xT, maskT, consts, s1d):
    nc = tc.nc
    add, mult, amax = mybir.AluOpType.add, mybir.AluOpType.mult, mybir.AluOpType.max

    const = ctx.enter_context(tc.tile_pool(name="const", bufs=1))
    xpool = ctx.enter_context(tc.tile_pool(name="xpool", bufs=NB))
    mpool = ctx.enter_context(tc.tile_pool(name="mpool", bufs=NB))
    spool = ctx.enter_context(tc.tile_pool(name="spool", bufs=NB))
    s2tpool = ctx.enter_context(tc.tile_pool(name="s2tpool", bufs=2 * NCH))
    vpool = ctx.enter_context(tc.tile_pool(name="vpool", bufs=4))
    upool = ctx.enter_context(tc.tile_pool(name="upool", bufs=4))
    epool = ctx.enter_context(tc.tile_pool(name="epool", bufs=4))
    s1bpool = ctx.enter_context(tc.tile_pool(name="s1bpool", bufs=6))
    hpool = ctx.enter_context(tc.tile_pool(name="hpool", bufs=2 * NCH))
    apool = ctx.enter_context(tc.tile_pool(name="apool", bufs=2))
    evpool = ctx.enter_context(tc.tile_pool(name="evpool", bufs=4))
    rpool = ctx.enter_context(tc.tile_pool(name="rpool", bufs=12))
    # PSUM: 2 (setup scratch) + 6 (agg out) = 8 banks
    pset = ctx.enter_context(tc.tile_pool(name="pset", bufs=2, space="PSUM"))
    pout = ctx.enter_context(tc.tile_pool(name="pout", bufs=6, space="PSUM"))

    # --- constants (one packed DMA) ----------------------------------------
    cst = const.tile([P, CONST_COLS], F16)
    nc.sync.dma_start(out=cst, in_=consts)
    W_sb = cst[:, C_W : C_W + FOUT]
    WT_sb = cst[:, C_WT : C_WT + FOUT]
    aT_sb = cst[:, C_AT : C_AT + 2 * H]
    ab_sb = cst[0:H, C_AB : C_AB + 1]
    ident8 = cst[0:H, C_ID8 : C_ID8 + H]
    onesrow = cst[0:1, C_ONES : C_ONES + P]

    # Wa[fin, 0:8]=W@a1^T, [fin, 8:16]=W@a2^T  (shared across graphs)
    p_wa = pset.tile([P, D], F32, tag="setup")
    nc.tensor.matmul(p_wa[:, 0 : 2 * H], WT_sb, aT_sb, start=True, stop=True)
    Wa_sb = const.tile([FIN, 2 * H], F16)
    nc.scalar.activation(Wa_sb[:], p_wa[:, 0 : 2 * H], mybir.ActivationFunctionType.Copy)

    G = []  # per-graph setup state
    for b in range(NB):
        # --- per-graph setup ----------------------------------------------
        x_sb = xpool.tile([FIN, D], F16, tag="x")
        nc.sync.dma_start(out=x_sb, in_=xT[b])
        m_sb = mpool.tile([P, NCH * D], F16, tag="mask")
        nc.gpsimd.dma_start(out=m_sb, in_=maskT[b])

        # s1/s2 for all heads: [8, D] each
        p_s1 = pset.tile([P, D], F32, tag="setup")
        nc.tensor.matmul(p_s1[0:H, :], Wa_sb[:, 0:H], x_sb[:], start=True, stop=True)
        s1_sb = spool.tile([H, D], F16, tag="s1")
        nc.scalar.activation(s1_sb[:], p_s1[0:H, :], mybir.ActivationFunctionType.Copy)
        # stage s1 rows in DRAM; the head loop row-broadcasts them back via DMA
        nc.scalar.dma_start(out=s1d[b], in_=s1_sb[:])
        mx1 = spool.tile([H, 1], F32, tag="mx1")
        nc.vector.reduce_max(
            out=mx1[:], in_=p_s1[0:H, :], axis=mybir.AxisListType.X, negate=True
        )

        p_s2 = pset.tile([P, D], F32, tag="setup")
        nc.tensor.matmul(
            p_s2[0:H, :], Wa_sb[:, H : 2 * H], x_sb[:], start=True, stop=True
        )
        s2b_sb = spool.tile([H, D], F16, tag="s2")
        nc.scalar.activation(
            s2b_sb[:], p_s2[0:H, :], mybir.ActivationFunctionType.Identity,
            bias=ab_sb,
        )
        mx2 = spool.tile([H, 1], F32, tag="mx2")
        nc.vector.reduce_max(
            out=mx2[:], in_=s2b_sb[:], axis=mybir.AxisListType.X, negate=True
        )

        # Per-head negated logit upper bound + 8: exp bias (softmax-shift
        # errors cancel per head, so fp16 is fine here).
        nbound = spool.tile([H, 1], F32, tag="nbound")
        nc.vector.tensor_add(nbound[:], mx1[:], mx2[:])
        nc.vector.tensor_scalar_add(nbound[:], nbound[:], 8.0)
        nb16 = spool.tile([H, 1], F16, tag="nb16")
        nc.vector.tensor_copy(nb16[:], nbound[:])
        # broadcast -bound to [P, H] columns: transpose to a row, then
        # ones-row outer-product
        p_nt = pset.tile([P, D], F32, tag="setup")
        nc.tensor.matmul(p_nt[0:1, 0:H], nb16[:], ident8, start=True, stop=True)
        nbT = spool.tile([1, H], F16, tag="nbT")
        nc.vector.tensor_copy(nbT[:], p_nt[0:1, 0:H])
        p_nb = pset.tile([P, D], F32, tag="setup")
        nc.tensor.matmul(p_nb[:, 0:H], onesrow, nbT[:], start=True, stop=True)
        nbcols = spool.tile([P, H], F16, tag="nbcols")
        nc.scalar.activation(
            nbcols[:], p_nb[:, 0:H], mybir.ActivationFunctionType.Copy
        )

        # s2b columns: [P, H] per j-chunk (PE transpose of [8, 128] slices)
        s2bT = []
        for c in range(NCH):
            p_t = pset.tile([P, 2 * D], F16, tag="setup")
            nc.tensor.transpose(p_t[:, 0:H], s2b_sb[:, bass.ts(c, P)], ident8)
            st = s2tpool.tile([P, H], F32, tag="s2T")
            nc.scalar.activation(st[:], p_t[:, 0:H], mybir.ActivationFunctionType.Copy)
            s2bT.append(st)

        # h tiles + ones column, fp16, h pre-scaled by 1/H
        haug = []
        for c in range(NCH):
            p_h = pset.tile([P, D], F32, tag="setup")
            nc.tensor.matmul(
                p_h[:, 0:FOUT], x_sb[:, bass.ts(c, P)], W_sb, start=True, stop=True
            )
            ha = hpool.tile([P, FOUT + 1], F16, tag="haug")
            nc.scalar.activation(
                ha[:, 0:FOUT], p_h[:, 0:FOUT],
                mybir.ActivationFunctionType.Copy, scale=1.0 / H,
            )
            nc.vector.memset(ha[:, FOUT : FOUT + 1], 1.0)
            haug.append(ha)

        acc = apool.tile([P, NCH * FOUT], F16, tag="acc")
        G.append(dict(m_sb=m_sb, s2bT=s2bT, haug=haug, acc=acc, nbcols=nbcols))

    # --- main per-head loop, graphs interleaved for deeper ILP ------------
    for hd in range(H):
        for b in range(NB):
            m_sb, s2bT = G[b]["m_sb"], G[b]["s2bT"]
            haug, acc, nbcols = G[b]["haug"], G[b]["acc"], G[b]["nbcols"]
            # S1B = s1 row hd broadcast across partitions (DMA row-bcast)
            s1b = s1bpool.tile([P, D], F16, tag="s1b")
            s1row = s1d[b, hd]
            nc.gpsimd.dma_start(
                out=s1b[:],
                in_=bass.AP(
                    tensor=s1d.tensor, offset=s1row.offset,
                    ap=[[0, P], s1row.ap[-1]],
                ),
            )

            # u = leaky_relu(maskT + s2b[j] + S1B): one fused custom DVE op
            # per j-chunk (replaces STT + separate lrelu pass)
            u = upool.tile([P, NCH * D], F16, tag="u")
            for c in range(NCH):
                nc.vector._custom_dve(
                    VLRELU,
                    out=u[:, bass.ts(c, D)],
                    in0=m_sb[:, bass.ts(c, D)],
                    in1=s1b[:],
                    s0=s2bT[c][:, hd : hd + 1],
                    imm2=0.01,
                )
            # E = exp(u - bound + 8), one wide fp16 ACT pass
            E = epool.tile([P, NCH * D], F16, tag="E")
            nc.scalar.activation(
                E[:], u[:], mybir.ActivationFunctionType.Exp,
                bias=nbcols[:, hd : hd + 1],
            )

            # agg: psum[i-tile t] += E^T[:, t]^T @ [h/8 | 1]
            p_os, rcols = [], []
            for t in range(NCH):
                p_o = pout.tile([P, FOUT + 1], F32, tag="po")
                for c in range(NCH):
                    nc.tensor.matmul(
                        p_o[:],
                        E[:, c * D + t * P : c * D + (t + 1) * P],
                        haug[c][:],
                        start=(c == 0),
                        stop=(c == NCH - 1),
                    )
                p_os.append(p_o)
            for t in range(NCH):
                rcol = rpool.tile([P, 1], F32, tag="rcol")
                nc.vector.reciprocal(rcol[:], p_os[t][:, FOUT : FOUT + 1])
                rcols.append(rcol)
            # normalize-evict on ACT (Copy with per-partition scale AP);
            # first head writes acc directly, later heads go via ev + one
            # wide fp16 DVE add.
            if hd == 0:
                for t in range(NCH):
                    nc.scalar.activation(
                        acc[:, bass.ts(t, FOUT)], p_os[t][:, 0:FOUT],
                        mybir.ActivationFunctionType.Copy, scale=rcols[t][:],
                    )
            else:
                ev = evpool.tile([P, NCH * FOUT], F16, tag="ev")
                for t in range(NCH):
                    nc.scalar.activation(
                        ev[:, bass.ts(t, FOUT)], p_os[t][:, 0:FOUT],
                        mybir.ActivationFunctionType.Copy, scale=rcols[t][:],
                    )
                # head-accumulate on GPSIMD (otherwise idle) to keep DVE free
                nc.gpsimd.tensor_add(acc[:], acc[:], ev[:])

    for b in range(NB):
        # out[b, t*128+p, f] = acc[p, t*FOUT+f]
        nc.sync.dma_start(
            out=bass.AP(
                tensor=out.tensor, offset=out[b].offset,
                ap=[[FOUT, P], [P * FOUT, NCH], [1, FOUT]],
            ),
            in_=G[b]["acc"][:],
        )


def _prep_core_inputs(input, adj, W, a_w, a_b, core):
    gs = slice(core * NB, (core + 1) * NB)
    x_c = input[gs]                                   # [NB, D, FIN]
    adj_c = adj[gs]                                   # [NB, D, D] int32
    xT = np.ascontiguousarray(x_c.transpose(0, 2, 1)).astype(np.float16)
    adjT = adj_c.transpose(0, 2, 1)                   # [NB, j, i]

    maskT = np.where(adjT > 0, np.float16(0.0), np.float16(NEGM))
    # [NB, j, i] -> [NB, NCH, P, i] -> [NB, P, NCH, i] -> [NB, P, NCH*D]
    maskT = np.ascontiguousarray(
        maskT.reshape(NB, NCH, P, D).transpose(0, 2, 1, 3).reshape(NB, P, NCH * D)
    )
    return {
        "xT": xT,
        "maskT": maskT,
        "consts": _pack_consts(W, a_w, a_b),
    }


def _pack_consts(W, a_w, a_b):
    c = np.zeros((P, CONST_COLS), dtype=np.float16)
    c[:, C_W : C_W + FOUT] = W
    c[:, C_WT : C_WT + FOUT] = W.T
    c[:, C_AT : C_AT + H] = a_w[:, :FOUT].T
    c[:, C_AT + H : C_AT + 2 * H] = a_w[:, FOUT:].T
    c[0:H, C_AB] = a_b
    c[0:H, C_ID8 : C_ID8 + H] = np.eye(H)
    c[0:1, C_ONES : C_ONES + P] = 1.0
    return c


def get_nc():
    if "nc" not in _NC_CACHE:
        _NC_CACHE["nc"] = _build_bass()
    return _NC_CACHE["nc"]


def run_on_device(in_maps, **kwargs):
    return run_bass_kernel_spmd(get_nc(), in_maps, list(range(NCORES)), **kwargs)


def kernel(input, adj, W, a_w, a_b):
    input = np.asarray(input, dtype=np.float32)
    adj = np.asarray(adj)
    W = np.asarray(W, dtype=np.float32)
    a_w = np.asarray(a_w, dtype=np.float32)
    a_b = np.asarray(a_b, dtype=np.float32)

    in_maps = [
        _prep_core_inputs(input, adj, W, a_w, a_b, c) for c in range(NCORES)
    ]
    res = run_on_device(in_maps)
    outs = [res.results[c]["out"] for c in range(NCORES)]
    return np.concatenate(outs, axis=0).astype(np.float32)


if __name__ == "__main__":
    nc = get_nc()
    print("built ok")
